# revision 10
# baseline (speedup 1.0000x reference)
"""Multi-head attention (B=2, S=2048, D=2048, H=16) on 8 Trainium2 NeuronCores.

Sharding: 2-way batch x 4-way head-group tensor parallelism. Core c handles
batch c//4 and heads [4*(c%4), 4*(c%4)+4). Each core:
  - projects its 4 heads' Q^T, K^T (head-dim-major) and V (row-major) with
    float32r matmuls (full PE rate at N>=512, ~1e-4 relative error),
  - runs softmax(QK^T/sqrt(dk))V per head with scores kept K-major so the
    PV contraction needs no transposes; exp on the scalar engine,
  - contracts its 4 heads' output slice with its Wo row-slice into a partial
    [S, D] output.
Host sums the 4 partials per batch and adds bo.

Inputs are fed pre-transposed (q/k/v as [D, S] per batch) so every DMA is a
large contiguous-run transfer and no on-chip transposes are needed anywhere.
"""

import os
import sys

for _p in ("/opt/trn_rl_repo", "/opt/pypackages"):
    if _p not in sys.path:
        sys.path.insert(0, _p)

import numpy as np

import concourse.bass as bass
import concourse.mybir as mybir
import concourse.tile as tile
from concourse import bacc
from concourse.bass_utils import run_bass_kernel_spmd

B = 2
S = 2048
D = 2048
H = 16
DK = 128
N_CORES = 8
HPC = 4          # heads per core
CW = HPC * DK    # per-core projection width = 512
P = 128
NRB = S // 512   # 512-row blocks
NDO = D // P     # contraction chunks
INV_SQRT_DK = 1.0 / float(np.sqrt(DK))

f32 = mybir.dt.float32
f32r = mybir.dt.float32r

_CACHE = {}
LAST_EXEC_NS = None
_ONES = np.ones((P, 512), dtype=np.float32)


def _build():
    nc = bacc.Bacc(None, target_bir_lowering=False, debug=False)

    qT = nc.declare_dram_parameter("qT", [D, S], f32r, isOutput=False)
    kT = nc.declare_dram_parameter("kT", [D, S], f32r, isOutput=False)
    vT = nc.declare_dram_parameter("vT", [D, S], f32r, isOutput=False)
    Wq = nc.declare_dram_parameter("Wq", [D, CW], f32r, isOutput=False)
    Wk = nc.declare_dram_parameter("Wk", [D, CW], f32r, isOutput=False)
    Wv = nc.declare_dram_parameter("Wv", [D, CW], f32r, isOutput=False)
    Wo = nc.declare_dram_parameter("Wo", [CW, D], f32r, isOutput=False)
    bq = nc.declare_dram_parameter("bq", [1, CW], f32r, isOutput=False)
    bk = nc.declare_dram_parameter("bk", [1, CW], f32r, isOutput=False)
    bv = nc.declare_dram_parameter("bv", [1, CW], f32r, isOutput=False)
    ones_d = nc.declare_dram_parameter("ones", [P, 512], f32r, isOutput=False)
    Y = nc.declare_dram_parameter("Y", [S, D], f32, isOutput=True)

    qT3 = qT.rearrange("(do di) s -> di do s", di=P)
    kT3 = kT.rearrange("(do di) s -> di do s", di=P)
    vT3 = vT.rearrange("(do di) s -> di do s", di=P)
    Wq3 = Wq.rearrange("(do di) c -> di do c", di=P)
    Wk3 = Wk.rearrange("(do di) c -> di do c", di=P)
    Wv3 = Wv.rearrange("(do di) c -> di do c", di=P)
    Wo3 = Wo.rearrange("(cc ci) e -> ci cc e", ci=P)

    with tile.TileContext(nc) as tc:
        with (
            tc.tile_pool(name="const", bufs=1) as cp,
            tc.tile_pool(name="qkv", bufs=1) as qkvp,
        ):
            ones_t = cp.tile([P, 512], f32r, tag="ones_t")
            nc.sync.dma_start(out=ones_t[:], in_=ones_d[:])
            ones_col = ones_t[:, 0:1]
            ones_row = ones_t[0:1, :]
            bq_t = cp.tile([1, CW], f32r, tag="bq")
            bk_t = cp.tile([1, CW], f32r, tag="bk")
            bv_t = cp.tile([1, CW], f32r, tag="bv")
            nc.sync.dma_start(out=bq_t[:], in_=bq[:])
            nc.sync.dma_start(out=bk_t[:], in_=bk[:])
            nc.sync.dma_start(out=bv_t[:], in_=bv[:])

            # Resident per-head projected tensors.
            Qt = [qkvp.tile([P, S], f32r, tag=f"qt{h}", name=f"qt{h}") for h in range(HPC)]
            Kt = [qkvp.tile([P, S], f32r, tag=f"kt{h}", name=f"kt{h}") for h in range(HPC)]
            Vt = [qkvp.tile([P, CW], f32r, tag=f"vt{rc}", name=f"vt{rc}") for rc in range(S // P)]

            # ---- Phase P: projections -------------------------------------
            # Per projection: its W slice is loaded once and stays resident;
            # the input strips stream through as per-do tiles released after
            # their 4 matmuls. The 4 heads' (or 4 r-chunks') accumulation
            # groups run interleaved across PSUM banks so strip tiles die
            # fast and PE never waits on a full-strip load.
            with (
                tc.tile_pool(name="xstrip", bufs=18) as xp,
                tc.tile_pool(name="wres", bufs=2) as wp,
                tc.tile_pool(name="pjps", bufs=8, space="PSUM") as pjps,
            ):
                for name, x3, w3, b_t in (
                    ("v", vT3, Wv3, bv_t),
                    ("k", kT3, Wk3, bk_t),
                    ("q", qT3, Wq3, bq_t),
                ):
                    wt = wp.tile([P, NDO, CW], f32r, tag="w", name=f"w_{name}")
                    for wc in range(4):
                        nc.sync.dma_start(
                            out=wt[:, wc * 4:(wc + 1) * 4, :],
                            in_=w3[:, wc * 4:(wc + 1) * 4, :],
                        )
                    for rb in range(NRB):
                        rs = slice(rb * 512, (rb + 1) * 512)
                        sdo = []
                        for do in range(NDO):
                            st = xp.tile([P, 512], f32r, tag="strip",
                                         name=f"strip{name}{rb}_{do}")
                            nc.sync.dma_start(out=st[:], in_=x3[:, do, rs])
                            sdo.append(st)
                        ps4 = []
                        for j in range(4):
                            ps = pjps.tile([P, 512], f32, tag="pj",
                                           name=f"pj{name}{rb}_{j}")
                            if name in ("q", "k"):
                                nc.tensor.matmul(
                                    ps[:], b_t[0:1, j * P:(j + 1) * P],
                                    ones_row, start=True, stop=False,
                                )
                            else:
                                nc.tensor.matmul(
                                    ps[:], ones_t[0:1, 0:P], b_t[:],
                                    start=True, stop=False,
                                )
                            ps4.append(ps)
                        for do in range(NDO):
                            for j in range(4):
                                if name in ("q", "k"):
                                    nc.tensor.matmul(
                                        ps4[j][:], wt[:, do, j * P:(j + 1) * P],
                                        sdo[do][:],
                                        start=False, stop=(do == NDO - 1),
                                    )
                                else:
                                    nc.tensor.matmul(
                                        ps4[j][:],
                                        sdo[do][:, j * P:(j + 1) * P],
                                        wt[:, do, :],
                                        start=False, stop=(do == NDO - 1),
                                    )
                        for j in range(4):
                            if name in ("q", "k"):
                                dst = Qt if name == "q" else Kt
                                nc.vector.tensor_copy(dst[j][:, rs], ps4[j][:])
                            else:
                                nc.vector.tensor_copy(Vt[rb * 4 + j][:], ps4[j][:])

            # ---- Phase A: attention + output projection -------------------
            with (
                tc.tile_pool(name="wo", bufs=1) as wop,
                tc.tile_pool(name="pt", bufs=8) as ptp,
                tc.tile_pool(name="ot", bufs=2) as otp,
                tc.tile_pool(name="nrm", bufs=2) as nrmp,
                tc.tile_pool(name="ystage", bufs=2) as yp,
                tc.tile_pool(name="sps", bufs=2, space="PSUM") as sps,
                tc.tile_pool(name="ops", bufs=3, space="PSUM") as ops,
                tc.tile_pool(name="dps", bufs=1, space="PSUM") as dps,
                tc.tile_pool(name="yps", bufs=2, space="PSUM") as yps,
            ):
                wo_t = []
                for cc in range(HPC):
                    t = wop.tile([P, D], f32r, tag=f"wo{cc}", name=f"wo{cc}")
                    nc.sync.dma_start(out=t[:], in_=Wo3[:, cc, :])
                    wo_t.append(t)

                for qb in range(NRB):
                    qs = slice(qb * 512, (qb + 1) * 512)
                    ot_tiles = []
                    for h in range(HPC):
                        ps_o = ops.tile([P, 512], f32, tag="o")
                        den = nrmp.tile([P, 512], f32r, tag="den")
                        den2 = nrmp.tile([P, 512], f32r, tag="den2")
                        for kc in range(S // P):
                            ps_s = sps.tile([P, 512], f32, tag="s")
                            nc.tensor.matmul(
                                ps_s[:], Kt[h][:, kc * P:(kc + 1) * P],
                                Qt[h][:, qs], start=True, stop=True,
                            )
                            pt = ptp.tile([P, 512], f32r, tag="pt")
                            nc.scalar.activation(
                                pt[:], ps_s[:],
                                mybir.ActivationFunctionType.Exp,
                                scale=INV_SQRT_DK,
                            )
                            nc.tensor.matmul(
                                ps_o[:], Vt[kc][:, h * P:(h + 1) * P], pt[:],
                                start=(kc == 0), stop=(kc == S // P - 1),
                            )
                            dtile = den if kc % 2 == 0 else den2
                            if kc < 2:
                                nc.vector.tensor_copy(dtile[:], pt[:])
                            else:
                                nc.vector.tensor_add(dtile[:], dtile[:], pt[:])
                        nc.vector.tensor_add(den[:], den[:], den2[:])
                        ps_d = dps.tile([1, 512], f32, tag="d")
                        nc.tensor.matmul(
                            ps_d[:], ones_col, den[:], start=True, stop=True,
                        )
                        recip = nrmp.tile([1, 512], f32, tag="recip")
                        rscr = nrmp.tile([1, 512], f32, tag="rscr")
                        nc.vector.reciprocal_approx_accurate(recip[:], ps_d[:], rscr[:])
                        rbc = nrmp.tile([P, 512], f32, tag="rbc")
                        nc.gpsimd.partition_broadcast(rbc[:], recip[:])
                        ot = otp.tile([P, 512], f32r, tag=f"ot{h}", name=f"ot{h}")
                        nc.vector.tensor_mul(ot[:], ps_o[:], rbc[:])
                        ot_tiles.append(ot)

                    for rc in range(4):
                        ytile = yp.tile([P, D], f32, tag="y")
                        for eb in range(4):
                            ps_y = yps.tile([P, 512], f32, tag="y")
                            for hc in range(HPC):
                                nc.tensor.matmul(
                                    ps_y[:],
                                    ot_tiles[hc][:, rc * P:(rc + 1) * P],
                                    wo_t[hc][:, eb * 512:(eb + 1) * 512],
                                    start=(hc == 0), stop=(hc == HPC - 1),
                                )
                            nc.vector.tensor_copy(
                                ytile[:, eb * 512:(eb + 1) * 512], ps_y[:]
                            )
                        row0 = qb * 512 + rc * P
                        nc.sync.dma_start(out=Y[row0:row0 + P, :], in_=ytile[:])

    nc.compile()
    return nc


def kernel(q, k, v, Wq, bq, Wk, bk, Wv, bv, Wo, bo):
    global LAST_EXEC_NS
    q = np.asarray(q, dtype=np.float32)
    k = np.asarray(k, dtype=np.float32)
    v = np.asarray(v, dtype=np.float32)
    Wq = np.asarray(Wq, dtype=np.float32)
    Wk = np.asarray(Wk, dtype=np.float32)
    Wv = np.asarray(Wv, dtype=np.float32)
    Wo = np.asarray(Wo, dtype=np.float32)
    bq = np.asarray(bq, dtype=np.float32)
    bk = np.asarray(bk, dtype=np.float32)
    bv = np.asarray(bv, dtype=np.float32)
    bo = np.asarray(bo, dtype=np.float32)

    if "nc" not in _CACHE:
        _CACHE["nc"] = _build()
    nc = _CACHE["nc"]

    qTs = [np.ascontiguousarray(q[b].T) for b in range(B)]
    kTs = [np.ascontiguousarray(k[b].T) for b in range(B)]
    vTs = [np.ascontiguousarray(v[b].T) for b in range(B)]

    in_maps = []
    for c in range(N_CORES):
        b = c // (N_CORES // B)
        g = c % (N_CORES // B)
        cs = slice(g * CW, (g + 1) * CW)
        in_maps.append({
            "qT": qTs[b], "kT": kTs[b], "vT": vTs[b],
            "Wq": np.ascontiguousarray(Wq[:, cs]),
            "Wk": np.ascontiguousarray(Wk[:, cs]),
            "Wv": np.ascontiguousarray(Wv[:, cs]),
            "Wo": np.ascontiguousarray(Wo[cs, :]),
            "bq": np.ascontiguousarray(bq[cs])[None, :],
            "bk": np.ascontiguousarray(bk[cs])[None, :],
            "bv": np.ascontiguousarray(bv[cs])[None, :],
            "ones": _ONES,
        })

    _CACHE["last_in_maps"] = in_maps
    res = run_bass_kernel_spmd(nc, in_maps, core_ids=list(range(N_CORES)))
    LAST_EXEC_NS = res.exec_time_ns

    gpb = N_CORES // B
    out = np.empty((B, S, D), dtype=np.float32)
    for b in range(B):
        acc = res.results[b * gpb]["Y"].astype(np.float32)
        for g in range(1, gpb):
            acc = acc + res.results[b * gpb + g]["Y"]
        out[b] = acc + bo[None, :]
    return out


# revision 11
# speedup vs baseline: 1.0437x; 1.0437x over previous
"""Multi-head attention (B=2, S=2048, D=2048, H=16) on 8 Trainium2 NeuronCores.

Sharding: 2-way batch x 4-way head-group tensor parallelism. Core c handles
batch c//4 and heads [4*(c%4), 4*(c%4)+4). Each core:
  - projects its 4 heads' Q^T, K^T (head-dim-major) and V (row-major) with
    float32r matmuls (full PE rate at N>=512, ~1e-4 relative error),
  - runs softmax(QK^T/sqrt(dk))V per head with scores kept K-major so the
    PV contraction needs no transposes; exp on the scalar engine,
  - contracts its 4 heads' output slice with its Wo row-slice into a partial
    [S, D] output.
Host sums the 4 partials per batch and adds bo.

Inputs are fed pre-transposed (q/k/v as [D, S] per batch) so every DMA is a
large contiguous-run transfer and no on-chip transposes are needed anywhere.
"""

import os
import sys

for _p in ("/opt/trn_rl_repo", "/opt/pypackages"):
    if _p not in sys.path:
        sys.path.insert(0, _p)

import numpy as np

import concourse.bass as bass
import concourse.mybir as mybir
import concourse.tile as tile
from concourse import bacc
from concourse.bass_utils import run_bass_kernel_spmd

B = 2
S = 2048
D = 2048
H = 16
DK = 128
N_CORES = 8
HPC = 4          # heads per core
CW = HPC * DK    # per-core projection width = 512
P = 128
NRB = S // 512   # 512-row blocks
NDO = D // P     # contraction chunks
INV_SQRT_DK = 1.0 / float(np.sqrt(DK))

f32 = mybir.dt.float32
f32r = mybir.dt.float32r

_CACHE = {}
LAST_EXEC_NS = None
_ONES = np.ones((P, 512), dtype=np.float32)


def _build():
    nc = bacc.Bacc(None, target_bir_lowering=False, debug=False)

    qT = nc.declare_dram_parameter("qT", [D, S], f32r, isOutput=False)
    kT = nc.declare_dram_parameter("kT", [D, S], f32r, isOutput=False)
    vT = nc.declare_dram_parameter("vT", [D, S], f32r, isOutput=False)
    Wq = nc.declare_dram_parameter("Wq", [D, CW], f32r, isOutput=False)
    Wk = nc.declare_dram_parameter("Wk", [D, CW], f32r, isOutput=False)
    Wv = nc.declare_dram_parameter("Wv", [D, CW], f32r, isOutput=False)
    Wo = nc.declare_dram_parameter("Wo", [CW, D], f32r, isOutput=False)
    bq = nc.declare_dram_parameter("bq", [1, CW], f32r, isOutput=False)
    bk = nc.declare_dram_parameter("bk", [1, CW], f32r, isOutput=False)
    bv = nc.declare_dram_parameter("bv", [1, CW], f32r, isOutput=False)
    ones_d = nc.declare_dram_parameter("ones", [P, 512], f32r, isOutput=False)
    Y = nc.declare_dram_parameter("Y", [S, D], f32, isOutput=True)

    qT3 = qT.rearrange("(do di) s -> di do s", di=P)
    kT3 = kT.rearrange("(do di) s -> di do s", di=P)
    vT3 = vT.rearrange("(do di) s -> di do s", di=P)
    Wq3 = Wq.rearrange("(do di) c -> di do c", di=P)
    Wk3 = Wk.rearrange("(do di) c -> di do c", di=P)
    Wv3 = Wv.rearrange("(do di) c -> di do c", di=P)
    Wo3 = Wo.rearrange("(cc ci) e -> ci cc e", ci=P)

    with tile.TileContext(nc) as tc:
        with (
            tc.tile_pool(name="const", bufs=1) as cp,
            tc.tile_pool(name="qkv", bufs=1) as qkvp,
        ):
            ones_t = cp.tile([P, 512], f32r, tag="ones_t")
            nc.sync.dma_start(out=ones_t[:], in_=ones_d[:])
            ones_col = ones_t[:, 0:1]
            ones_row = ones_t[0:1, :]
            bq_t = cp.tile([1, CW], f32r, tag="bq")
            bk_t = cp.tile([1, CW], f32r, tag="bk")
            bv_t = cp.tile([1, CW], f32r, tag="bv")
            nc.sync.dma_start(out=bq_t[:], in_=bq[:])
            nc.sync.dma_start(out=bk_t[:], in_=bk[:])
            nc.sync.dma_start(out=bv_t[:], in_=bv[:])

            # Resident per-head projected tensors.
            Qt = [qkvp.tile([P, S], f32r, tag=f"qt{h}", name=f"qt{h}") for h in range(HPC)]
            Kt = [qkvp.tile([P, S], f32r, tag=f"kt{h}", name=f"kt{h}") for h in range(HPC)]
            Vt = [qkvp.tile([P, CW], f32r, tag=f"vt{rc}", name=f"vt{rc}") for rc in range(S // P)]

            # ---- Phase P: projections -------------------------------------
            # Per projection: its W slice is loaded once and stays resident;
            # the input strips stream through as per-do tiles released after
            # their 4 matmuls. The 4 heads' (or 4 r-chunks') accumulation
            # groups run interleaved across PSUM banks so strip tiles die
            # fast and PE never waits on a full-strip load.
            with (
                tc.tile_pool(name="xstrip", bufs=18) as xp,
                tc.tile_pool(name="wres", bufs=2) as wp,
                tc.tile_pool(name="pjps", bufs=8, space="PSUM") as pjps,
            ):
                for name, x3, w3, b_t in (
                    ("v", vT3, Wv3, bv_t),
                    ("k", kT3, Wk3, bk_t),
                    ("q", qT3, Wq3, bq_t),
                ):
                    wt = wp.tile([P, NDO, CW], f32r, tag="w", name=f"w_{name}")
                    for wc in range(4):
                        nc.sync.dma_start(
                            out=wt[:, wc * 4:(wc + 1) * 4, :],
                            in_=w3[:, wc * 4:(wc + 1) * 4, :],
                        )
                    for rb in range(NRB):
                        rs = slice(rb * 512, (rb + 1) * 512)
                        sdo = []
                        for do in range(NDO):
                            st = xp.tile([P, 512], f32r, tag="strip",
                                         name=f"strip{name}{rb}_{do}")
                            nc.sync.dma_start(out=st[:], in_=x3[:, do, rs])
                            sdo.append(st)
                        ps4 = []
                        for j in range(4):
                            ps = pjps.tile([P, 512], f32, tag="pj",
                                           name=f"pj{name}{rb}_{j}")
                            if name in ("q", "k"):
                                nc.tensor.matmul(
                                    ps[:], b_t[0:1, j * P:(j + 1) * P],
                                    ones_row, start=True, stop=False,
                                )
                            else:
                                nc.tensor.matmul(
                                    ps[:], ones_t[0:1, 0:P], b_t[:],
                                    start=True, stop=False,
                                )
                            ps4.append(ps)
                        for do in range(NDO):
                            for j in range(4):
                                if name in ("q", "k"):
                                    nc.tensor.matmul(
                                        ps4[j][:], wt[:, do, j * P:(j + 1) * P],
                                        sdo[do][:],
                                        start=False, stop=(do == NDO - 1),
                                    )
                                else:
                                    nc.tensor.matmul(
                                        ps4[j][:],
                                        sdo[do][:, j * P:(j + 1) * P],
                                        wt[:, do, :],
                                        start=False, stop=(do == NDO - 1),
                                    )
                        for j in range(4):
                            if name in ("q", "k"):
                                dst = Qt if name == "q" else Kt
                                nc.vector.tensor_copy(dst[j][:, rs], ps4[j][:])
                            else:
                                nc.vector.tensor_copy(Vt[rb * 4 + j][:], ps4[j][:])

            # ---- Phase A: attention + output projection -------------------
            with (
                tc.tile_pool(name="wo", bufs=1) as wop,
                tc.tile_pool(name="pt", bufs=8) as ptp,
                tc.tile_pool(name="ot", bufs=2) as otp,
                tc.tile_pool(name="nrm", bufs=2) as nrmp,
                tc.tile_pool(name="ystage", bufs=2) as yp,
                tc.tile_pool(name="sps", bufs=3, space="PSUM") as sps,
                tc.tile_pool(name="ops", bufs=2, space="PSUM") as ops,
                tc.tile_pool(name="dps", bufs=1, space="PSUM") as dps,
                tc.tile_pool(name="yps", bufs=2, space="PSUM") as yps,
            ):
                wo_t = []
                for cc in range(HPC):
                    t = wop.tile([P, D], f32r, tag=f"wo{cc}", name=f"wo{cc}")
                    nc.sync.dma_start(out=t[:], in_=Wo3[:, cc, :])
                    wo_t.append(t)

                for qb in range(NRB):
                    qs = slice(qb * 512, (qb + 1) * 512)
                    ot_tiles = []
                    for h in range(HPC):
                        ps_o = ops.tile([P, 512], f32, tag="o")
                        den = nrmp.tile([P, 512], f32r, tag="den")
                        den2 = nrmp.tile([P, 512], f32r, tag="den2")
                        for kc in range(S // P):
                            ps_s = sps.tile([P, 512], f32, tag="s")
                            nc.tensor.matmul(
                                ps_s[:], Kt[h][:, kc * P:(kc + 1) * P],
                                Qt[h][:, qs], start=True, stop=True,
                            )
                            pt = ptp.tile([P, 512], f32r, tag="pt")
                            nc.scalar.activation(
                                pt[:], ps_s[:],
                                mybir.ActivationFunctionType.Exp,
                                scale=INV_SQRT_DK,
                            )
                            nc.tensor.matmul(
                                ps_o[:], Vt[kc][:, h * P:(h + 1) * P], pt[:],
                                start=(kc == 0), stop=(kc == S // P - 1),
                            )
                            dtile = den if kc % 2 == 0 else den2
                            if kc < 2:
                                nc.vector.tensor_copy(dtile[:], pt[:])
                            else:
                                nc.vector.tensor_add(dtile[:], dtile[:], pt[:])
                        nc.vector.tensor_add(den[:], den[:], den2[:])
                        ps_d = dps.tile([1, 512], f32, tag="d")
                        nc.tensor.matmul(
                            ps_d[:], ones_col, den[:], start=True, stop=True,
                        )
                        recip = nrmp.tile([1, 512], f32, tag="recip")
                        rscr = nrmp.tile([1, 512], f32, tag="rscr")
                        nc.vector.reciprocal_approx_accurate(recip[:], ps_d[:], rscr[:])
                        rbc = nrmp.tile([P, 512], f32, tag="rbc")
                        nc.gpsimd.partition_broadcast(rbc[:], recip[:])
                        ot = otp.tile([P, 512], f32r, tag=f"ot{h}", name=f"ot{h}")
                        nc.vector.tensor_mul(ot[:], ps_o[:], rbc[:])
                        ot_tiles.append(ot)

                    for rc in range(4):
                        ytile = yp.tile([P, D], f32, tag="y")
                        for eb in range(4):
                            ps_y = yps.tile([P, 512], f32, tag="y")
                            for hc in range(HPC):
                                nc.tensor.matmul(
                                    ps_y[:],
                                    ot_tiles[hc][:, rc * P:(rc + 1) * P],
                                    wo_t[hc][:, eb * 512:(eb + 1) * 512],
                                    start=(hc == 0), stop=(hc == HPC - 1),
                                )
                            nc.vector.tensor_copy(
                                ytile[:, eb * 512:(eb + 1) * 512], ps_y[:]
                            )
                        row0 = qb * 512 + rc * P
                        nc.sync.dma_start(out=Y[row0:row0 + P, :], in_=ytile[:])

    nc.compile()
    return nc


def kernel(q, k, v, Wq, bq, Wk, bk, Wv, bv, Wo, bo):
    global LAST_EXEC_NS
    q = np.asarray(q, dtype=np.float32)
    k = np.asarray(k, dtype=np.float32)
    v = np.asarray(v, dtype=np.float32)
    Wq = np.asarray(Wq, dtype=np.float32)
    Wk = np.asarray(Wk, dtype=np.float32)
    Wv = np.asarray(Wv, dtype=np.float32)
    Wo = np.asarray(Wo, dtype=np.float32)
    bq = np.asarray(bq, dtype=np.float32)
    bk = np.asarray(bk, dtype=np.float32)
    bv = np.asarray(bv, dtype=np.float32)
    bo = np.asarray(bo, dtype=np.float32)

    if "nc" not in _CACHE:
        _CACHE["nc"] = _build()
    nc = _CACHE["nc"]

    qTs = [np.ascontiguousarray(q[b].T) for b in range(B)]
    kTs = [np.ascontiguousarray(k[b].T) for b in range(B)]
    vTs = [np.ascontiguousarray(v[b].T) for b in range(B)]

    in_maps = []
    for c in range(N_CORES):
        b = c // (N_CORES // B)
        g = c % (N_CORES // B)
        cs = slice(g * CW, (g + 1) * CW)
        in_maps.append({
            "qT": qTs[b], "kT": kTs[b], "vT": vTs[b],
            "Wq": np.ascontiguousarray(Wq[:, cs]),
            "Wk": np.ascontiguousarray(Wk[:, cs]),
            "Wv": np.ascontiguousarray(Wv[:, cs]),
            "Wo": np.ascontiguousarray(Wo[cs, :]),
            "bq": np.ascontiguousarray(bq[cs])[None, :],
            "bk": np.ascontiguousarray(bk[cs])[None, :],
            "bv": np.ascontiguousarray(bv[cs])[None, :],
            "ones": _ONES,
        })

    _CACHE["last_in_maps"] = in_maps
    res = run_bass_kernel_spmd(nc, in_maps, core_ids=list(range(N_CORES)))
    LAST_EXEC_NS = res.exec_time_ns

    gpb = N_CORES // B
    out = np.empty((B, S, D), dtype=np.float32)
    for b in range(B):
        acc = res.results[b * gpb]["Y"].astype(np.float32)
        for g in range(1, gpb):
            acc = acc + res.results[b * gpb + g]["Y"]
        out[b] = acc + bo[None, :]
    return out


# revision 12
# speedup vs baseline: 1.1146x; 1.0680x over previous
"""Multi-head attention (B=2, S=2048, D=2048, H=16) on 8 Trainium2 NeuronCores.

Sharding: 2-way batch x 4-way head-group tensor parallelism. Core c handles
batch c//4 and heads [4*(c%4), 4*(c%4)+4). Each core:
  - projects its 4 heads' Q^T, K^T (head-dim-major) and V (row-major) with
    float32r matmuls (full PE rate at N>=512, ~1e-4 relative error),
  - runs softmax(QK^T/sqrt(dk))V per head with scores kept K-major so the
    PV contraction needs no transposes; exp on the scalar engine,
  - contracts its 4 heads' output slice with its Wo row-slice into a partial
    [S, D] output.
Host sums the 4 partials per batch and adds bo.

Inputs are fed pre-transposed (q/k/v as [D, S] per batch) so every DMA is a
large contiguous-run transfer and no on-chip transposes are needed anywhere.
"""

import os
import sys

for _p in ("/opt/trn_rl_repo", "/opt/pypackages"):
    if _p not in sys.path:
        sys.path.insert(0, _p)

import numpy as np

import concourse.bass as bass
import concourse.mybir as mybir
import concourse.tile as tile
from concourse import bacc
from concourse.bass_utils import run_bass_kernel_spmd

B = 2
S = 2048
D = 2048
H = 16
DK = 128
N_CORES = 8
HPC = 4          # heads per core
CW = HPC * DK    # per-core projection width = 512
P = 128
NRB = S // 512   # 512-row blocks
NDO = D // P     # contraction chunks
INV_SQRT_DK = 1.0 / float(np.sqrt(DK))

f32 = mybir.dt.float32
f32r = mybir.dt.float32r

_CACHE = {}
LAST_EXEC_NS = None
_ONES = np.ones((P, 512), dtype=np.float32)


def _build():
    nc = bacc.Bacc(None, target_bir_lowering=False, debug=False)

    qT = nc.declare_dram_parameter("qT", [D, S], f32r, isOutput=False)
    kT = nc.declare_dram_parameter("kT", [D, S], f32r, isOutput=False)
    vT = nc.declare_dram_parameter("vT", [D, S], f32r, isOutput=False)
    Wq = nc.declare_dram_parameter("Wq", [D, CW], f32r, isOutput=False)
    Wk = nc.declare_dram_parameter("Wk", [D, CW], f32r, isOutput=False)
    Wv = nc.declare_dram_parameter("Wv", [D, CW], f32r, isOutput=False)
    Wo = nc.declare_dram_parameter("Wo", [CW, D], f32r, isOutput=False)
    bq = nc.declare_dram_parameter("bq", [1, CW], f32r, isOutput=False)
    bk = nc.declare_dram_parameter("bk", [1, CW], f32r, isOutput=False)
    bv = nc.declare_dram_parameter("bv", [1, CW], f32r, isOutput=False)
    ones_d = nc.declare_dram_parameter("ones", [P, 512], f32r, isOutput=False)
    Y = nc.declare_dram_parameter("Y", [S, D], f32, isOutput=True)

    qT3 = qT.rearrange("(do di) s -> di do s", di=P)
    kT3 = kT.rearrange("(do di) s -> di do s", di=P)
    vT3 = vT.rearrange("(do di) s -> di do s", di=P)
    Wq3 = Wq.rearrange("(do di) c -> di do c", di=P)
    Wk3 = Wk.rearrange("(do di) c -> di do c", di=P)
    Wv3 = Wv.rearrange("(do di) c -> di do c", di=P)
    Wo3 = Wo.rearrange("(cc ci) e -> ci cc e", ci=P)

    with tile.TileContext(nc) as tc:
        with (
            tc.tile_pool(name="const", bufs=1) as cp,
            tc.tile_pool(name="qkv", bufs=1) as qkvp,
        ):
            ones_t = cp.tile([P, 512], f32r, tag="ones_t")
            nc.sync.dma_start(out=ones_t[:], in_=ones_d[:])
            ones_col = ones_t[:, 0:1]
            ones_row = ones_t[0:1, :]
            bq_t = cp.tile([1, CW], f32r, tag="bq")
            bk_t = cp.tile([1, CW], f32r, tag="bk")
            bv_t = cp.tile([1, CW], f32r, tag="bv")
            nc.sync.dma_start(out=bq_t[:], in_=bq[:])
            nc.sync.dma_start(out=bk_t[:], in_=bk[:])
            nc.sync.dma_start(out=bv_t[:], in_=bv[:])

            # Resident per-head projected tensors.
            Qt = [qkvp.tile([P, S], f32r, tag=f"qt{h}", name=f"qt{h}") for h in range(HPC)]
            Kt = [qkvp.tile([P, S], f32r, tag=f"kt{h}", name=f"kt{h}") for h in range(HPC)]
            Vt = [qkvp.tile([P, CW], f32r, tag=f"vt{rc}", name=f"vt{rc}") for rc in range(S // P)]

            # ---- Phase P: projections -------------------------------------
            # Per projection: its W slice is loaded once and stays resident;
            # the input strips stream through as per-do tiles released after
            # their 4 matmuls. The 4 heads' (or 4 r-chunks') accumulation
            # groups run interleaved across PSUM banks so strip tiles die
            # fast and PE never waits on a full-strip load.
            with (
                tc.tile_pool(name="xstrip", bufs=18) as xp,
                tc.tile_pool(name="wres", bufs=2) as wp,
                tc.tile_pool(name="pjps", bufs=8, space="PSUM") as pjps,
            ):
                for name, x3, w3, b_t in (
                    ("v", vT3, Wv3, bv_t),
                    ("k", kT3, Wk3, bk_t),
                    ("q", qT3, Wq3, bq_t),
                ):
                    wt = wp.tile([P, NDO, CW], f32r, tag="w", name=f"w_{name}")
                    for rb in range(NRB):
                        rs = slice(rb * 512, (rb + 1) * 512)
                        sdo = []
                        for do in range(NDO):
                            if rb == 0 and do % 4 == 0:
                                wc = do // 4
                                nc.sync.dma_start(
                                    out=wt[:, wc * 4:(wc + 1) * 4, :],
                                    in_=w3[:, wc * 4:(wc + 1) * 4, :],
                                )
                            st = xp.tile([P, 512], f32r, tag="strip",
                                         name=f"strip{name}{rb}_{do}")
                            nc.sync.dma_start(out=st[:], in_=x3[:, do, rs])
                            sdo.append(st)
                        ps4 = []
                        for j in range(4):
                            ps = pjps.tile([P, 512], f32, tag="pj",
                                           name=f"pj{name}{rb}_{j}")
                            if name in ("q", "k"):
                                nc.tensor.matmul(
                                    ps[:], b_t[0:1, j * P:(j + 1) * P],
                                    ones_row, start=True, stop=False,
                                )
                            else:
                                nc.tensor.matmul(
                                    ps[:], ones_t[0:1, 0:P], b_t[:],
                                    start=True, stop=False,
                                )
                            ps4.append(ps)
                        for do in range(NDO):
                            for j in range(4):
                                if name in ("q", "k"):
                                    nc.tensor.matmul(
                                        ps4[j][:], wt[:, do, j * P:(j + 1) * P],
                                        sdo[do][:],
                                        start=False, stop=(do == NDO - 1),
                                    )
                                else:
                                    nc.tensor.matmul(
                                        ps4[j][:],
                                        sdo[do][:, j * P:(j + 1) * P],
                                        wt[:, do, :],
                                        start=False, stop=(do == NDO - 1),
                                    )
                        for j in range(4):
                            if name in ("q", "k"):
                                dst = Qt if name == "q" else Kt
                                nc.vector.tensor_copy(dst[j][:, rs], ps4[j][:])
                            else:
                                nc.vector.tensor_copy(Vt[rb * 4 + j][:], ps4[j][:])

            # ---- Phase A: attention + output projection -------------------
            with (
                tc.tile_pool(name="wo", bufs=1) as wop,
                tc.tile_pool(name="pt", bufs=8) as ptp,
                tc.tile_pool(name="ot", bufs=2) as otp,
                tc.tile_pool(name="nrm", bufs=2) as nrmp,
                tc.tile_pool(name="ystage", bufs=2) as yp,
                tc.tile_pool(name="sps", bufs=3, space="PSUM") as sps,
                tc.tile_pool(name="ops", bufs=2, space="PSUM") as ops,
                tc.tile_pool(name="dps", bufs=1, space="PSUM") as dps,
                tc.tile_pool(name="yps", bufs=2, space="PSUM") as yps,
            ):
                wo_t = []
                for cc in range(HPC):
                    t = wop.tile([P, D], f32r, tag=f"wo{cc}", name=f"wo{cc}")
                    nc.sync.dma_start(out=t[:], in_=Wo3[:, cc, :])
                    wo_t.append(t)

                for qb in range(NRB):
                    qs = slice(qb * 512, (qb + 1) * 512)
                    ot_tiles = []
                    for h in range(HPC):
                        ps_o = ops.tile([P, 512], f32, tag="o")
                        den = nrmp.tile([P, 512], f32r, tag="den")
                        den2 = nrmp.tile([P, 512], f32r, tag="den2")
                        for kc in range(S // P):
                            ps_s = sps.tile([P, 512], f32, tag="s")
                            nc.tensor.matmul(
                                ps_s[:], Kt[h][:, kc * P:(kc + 1) * P],
                                Qt[h][:, qs], start=True, stop=True,
                            )
                            pt = ptp.tile([P, 512], f32r, tag="pt")
                            nc.scalar.activation(
                                pt[:], ps_s[:],
                                mybir.ActivationFunctionType.Exp,
                                scale=INV_SQRT_DK,
                            )
                            nc.tensor.matmul(
                                ps_o[:], Vt[kc][:, h * P:(h + 1) * P], pt[:],
                                start=(kc == 0), stop=(kc == S // P - 1),
                            )
                            dtile = den if kc % 2 == 0 else den2
                            if kc < 2:
                                nc.vector.tensor_copy(dtile[:], pt[:])
                            else:
                                nc.vector.tensor_add(dtile[:], dtile[:], pt[:])
                        nc.vector.tensor_add(den[:], den[:], den2[:])
                        ps_d = dps.tile([P, 512], f32, tag="d")
                        nc.tensor.matmul(
                            ps_d[:], ones_t[:, 0:P], den[:], start=True, stop=True,
                        )
                        rbc = nrmp.tile([P, 512], f32, tag="rbc")
                        nc.vector.reciprocal_approx_fast(rbc[:], ps_d[:])
                        ot = otp.tile([P, 512], f32r, tag=f"ot{h}", name=f"ot{h}")
                        nc.vector.tensor_mul(ot[:], ps_o[:], rbc[:])
                        ot_tiles.append(ot)

                    for rc in range(4):
                        ytile = yp.tile([P, D], f32, tag="y")
                        for eb in range(4):
                            ps_y = yps.tile([P, 512], f32, tag="y")
                            for hc in range(HPC):
                                nc.tensor.matmul(
                                    ps_y[:],
                                    ot_tiles[hc][:, rc * P:(rc + 1) * P],
                                    wo_t[hc][:, eb * 512:(eb + 1) * 512],
                                    start=(hc == 0), stop=(hc == HPC - 1),
                                )
                            nc.vector.tensor_copy(
                                ytile[:, eb * 512:(eb + 1) * 512], ps_y[:]
                            )
                        row0 = qb * 512 + rc * P
                        nc.sync.dma_start(out=Y[row0:row0 + P, :], in_=ytile[:])

    nc.compile()
    return nc


def kernel(q, k, v, Wq, bq, Wk, bk, Wv, bv, Wo, bo):
    global LAST_EXEC_NS
    q = np.asarray(q, dtype=np.float32)
    k = np.asarray(k, dtype=np.float32)
    v = np.asarray(v, dtype=np.float32)
    Wq = np.asarray(Wq, dtype=np.float32)
    Wk = np.asarray(Wk, dtype=np.float32)
    Wv = np.asarray(Wv, dtype=np.float32)
    Wo = np.asarray(Wo, dtype=np.float32)
    bq = np.asarray(bq, dtype=np.float32)
    bk = np.asarray(bk, dtype=np.float32)
    bv = np.asarray(bv, dtype=np.float32)
    bo = np.asarray(bo, dtype=np.float32)

    if "nc" not in _CACHE:
        _CACHE["nc"] = _build()
    nc = _CACHE["nc"]

    qTs = [np.ascontiguousarray(q[b].T) for b in range(B)]
    kTs = [np.ascontiguousarray(k[b].T) for b in range(B)]
    vTs = [np.ascontiguousarray(v[b].T) for b in range(B)]

    in_maps = []
    for c in range(N_CORES):
        b = c // (N_CORES // B)
        g = c % (N_CORES // B)
        cs = slice(g * CW, (g + 1) * CW)
        in_maps.append({
            "qT": qTs[b], "kT": kTs[b], "vT": vTs[b],
            "Wq": np.ascontiguousarray(Wq[:, cs]),
            "Wk": np.ascontiguousarray(Wk[:, cs]),
            "Wv": np.ascontiguousarray(Wv[:, cs]),
            "Wo": np.ascontiguousarray(Wo[cs, :]),
            "bq": np.ascontiguousarray(bq[cs])[None, :],
            "bk": np.ascontiguousarray(bk[cs])[None, :],
            "bv": np.ascontiguousarray(bv[cs])[None, :],
            "ones": _ONES,
        })

    _CACHE["last_in_maps"] = in_maps
    res = run_bass_kernel_spmd(nc, in_maps, core_ids=list(range(N_CORES)))
    LAST_EXEC_NS = res.exec_time_ns

    gpb = N_CORES // B
    out = np.empty((B, S, D), dtype=np.float32)
    for b in range(B):
        acc = res.results[b * gpb]["Y"].astype(np.float32)
        for g in range(1, gpb):
            acc = acc + res.results[b * gpb + g]["Y"]
        out[b] = acc + bo[None, :]
    return out


# revision 13
# speedup vs baseline: 1.1511x; 1.0328x over previous
"""Multi-head attention (B=2, S=2048, D=2048, H=16) on 8 Trainium2 NeuronCores.

Sharding: 2-way batch x 4-way head-group tensor parallelism. Core c handles
batch c//4 and heads [4*(c%4), 4*(c%4)+4). Each core:
  - projects its 4 heads' Q^T, K^T (head-dim-major) and V (row-major) with
    float32r matmuls (full PE rate at N>=512, ~1e-4 relative error),
  - runs softmax(QK^T/sqrt(dk))V per head with scores kept K-major so the
    PV contraction needs no transposes; exp on the scalar engine,
  - contracts its 4 heads' output slice with its Wo row-slice into a partial
    [S, D] output.
Host sums the 4 partials per batch and adds bo.

Inputs are fed pre-transposed (q/k/v as [D, S] per batch) so every DMA is a
large contiguous-run transfer and no on-chip transposes are needed anywhere.
"""

import os
import sys

for _p in ("/opt/trn_rl_repo", "/opt/pypackages"):
    if _p not in sys.path:
        sys.path.insert(0, _p)

import numpy as np

import concourse.bass as bass
import concourse.mybir as mybir
import concourse.tile as tile
from concourse import bacc
from concourse.bass_utils import run_bass_kernel_spmd

B = 2
S = 2048
D = 2048
H = 16
DK = 128
N_CORES = 8
HPC = 4          # heads per core
CW = HPC * DK    # per-core projection width = 512
P = 128
NRB = S // 512   # 512-row blocks
NDO = D // P     # contraction chunks
INV_SQRT_DK = 1.0 / float(np.sqrt(DK))

f32 = mybir.dt.float32
f32r = mybir.dt.float32r

_CACHE = {}
LAST_EXEC_NS = None
_ONES = np.ones((P, 512), dtype=np.float32)


def _build():
    nc = bacc.Bacc(None, target_bir_lowering=False, debug=False)

    qT = nc.declare_dram_parameter("qT", [D, S], f32r, isOutput=False)
    kT = nc.declare_dram_parameter("kT", [D, S], f32r, isOutput=False)
    vT = nc.declare_dram_parameter("vT", [D, S], f32r, isOutput=False)
    Wq = nc.declare_dram_parameter("Wq", [D, CW], f32r, isOutput=False)
    Wk = nc.declare_dram_parameter("Wk", [D, CW], f32r, isOutput=False)
    Wv = nc.declare_dram_parameter("Wv", [D, CW], f32r, isOutput=False)
    Wo = nc.declare_dram_parameter("Wo", [CW, D], f32r, isOutput=False)
    bq = nc.declare_dram_parameter("bq", [1, CW], f32r, isOutput=False)
    bk = nc.declare_dram_parameter("bk", [1, CW], f32r, isOutput=False)
    bv = nc.declare_dram_parameter("bv", [1, CW], f32r, isOutput=False)
    ones_d = nc.declare_dram_parameter("ones", [P, 512], f32r, isOutput=False)
    Y = nc.declare_dram_parameter("Y", [S, D], f32, isOutput=True)

    qT3 = qT.rearrange("(do di) s -> di do s", di=P)
    kT3 = kT.rearrange("(do di) s -> di do s", di=P)
    vT3 = vT.rearrange("(do di) s -> di do s", di=P)
    Wq3 = Wq.rearrange("(do di) c -> di do c", di=P)
    Wk3 = Wk.rearrange("(do di) c -> di do c", di=P)
    Wv3 = Wv.rearrange("(do di) c -> di do c", di=P)
    Wo3 = Wo.rearrange("(cc ci) e -> ci cc e", ci=P)

    with tile.TileContext(nc) as tc:
        with (
            tc.tile_pool(name="const", bufs=1) as cp,
            tc.tile_pool(name="qkv", bufs=1) as qkvp,
        ):
            ones_t = cp.tile([P, 512], f32r, tag="ones_t")
            nc.sync.dma_start(out=ones_t[:], in_=ones_d[:])
            ones_col = ones_t[:, 0:1]
            ones_row = ones_t[0:1, :]
            bq_t = cp.tile([1, CW], f32r, tag="bq")
            bk_t = cp.tile([1, CW], f32r, tag="bk")
            bv_t = cp.tile([1, CW], f32r, tag="bv")
            nc.sync.dma_start(out=bq_t[:], in_=bq[:])
            nc.sync.dma_start(out=bk_t[:], in_=bk[:])
            nc.sync.dma_start(out=bv_t[:], in_=bv[:])

            # Resident per-head projected tensors.
            Qt = [qkvp.tile([P, S], f32r, tag=f"qt{h}", name=f"qt{h}") for h in range(HPC)]
            Kt = [qkvp.tile([P, S], f32r, tag=f"kt{h}", name=f"kt{h}") for h in range(HPC)]
            Vt = [qkvp.tile([P, CW], f32r, tag=f"vt{rc}", name=f"vt{rc}") for rc in range(S // P)]

            # ---- Phase P: projections -------------------------------------
            # Per projection: its W slice is loaded once and stays resident;
            # the input strips stream through as per-do tiles released after
            # their 4 matmuls. The 4 heads' (or 4 r-chunks') accumulation
            # groups run interleaved across PSUM banks so strip tiles die
            # fast and PE never waits on a full-strip load.
            with (
                tc.tile_pool(name="xstrip", bufs=18) as xp,
                tc.tile_pool(name="wres", bufs=2) as wp,
                tc.tile_pool(name="pjps", bufs=8, space="PSUM") as pjps,
            ):
                for name, x3, w3, b_t in (
                    ("v", vT3, Wv3, bv_t),
                    ("k", kT3, Wk3, bk_t),
                    ("q", qT3, Wq3, bq_t),
                ):
                    wt = wp.tile([P, NDO, CW], f32r, tag="w", name=f"w_{name}")
                    for rb in range(NRB):
                        rs = slice(rb * 512, (rb + 1) * 512)
                        sdo = []
                        for do in range(NDO):
                            if rb == 0 and do % 4 == 0:
                                wc = do // 4
                                nc.sync.dma_start(
                                    out=wt[:, wc * 4:(wc + 1) * 4, :],
                                    in_=w3[:, wc * 4:(wc + 1) * 4, :],
                                )
                            st = xp.tile([P, 512], f32r, tag="strip",
                                         name=f"strip{name}{rb}_{do}")
                            nc.sync.dma_start(out=st[:], in_=x3[:, do, rs])
                            sdo.append(st)
                        ps4 = []
                        for j in range(4):
                            ps = pjps.tile([P, 512], f32, tag="pj",
                                           name=f"pj{name}{rb}_{j}")
                            if name in ("q", "k"):
                                nc.tensor.matmul(
                                    ps[:], b_t[0:1, j * P:(j + 1) * P],
                                    ones_row, start=True, stop=False,
                                )
                            else:
                                nc.tensor.matmul(
                                    ps[:], ones_t[0:1, 0:P], b_t[:],
                                    start=True, stop=False,
                                )
                            ps4.append(ps)
                        for do in range(NDO):
                            for j in range(4):
                                if name in ("q", "k"):
                                    nc.tensor.matmul(
                                        ps4[j][:], wt[:, do, j * P:(j + 1) * P],
                                        sdo[do][:],
                                        start=False, stop=(do == NDO - 1),
                                    )
                                else:
                                    nc.tensor.matmul(
                                        ps4[j][:],
                                        sdo[do][:, j * P:(j + 1) * P],
                                        wt[:, do, :],
                                        start=False, stop=(do == NDO - 1),
                                    )
                        for j in range(4):
                            if name in ("q", "k"):
                                dst = Qt if name == "q" else Kt
                                nc.vector.tensor_copy(dst[j][:, rs], ps4[j][:])
                            else:
                                nc.vector.tensor_copy(Vt[rb * 4 + j][:], ps4[j][:])

            # ---- Phase A: attention + output projection -------------------
            with (
                tc.tile_pool(name="wo", bufs=1) as wop,
                tc.tile_pool(name="pt", bufs=8) as ptp,
                tc.tile_pool(name="ot", bufs=2) as otp,
                tc.tile_pool(name="nrm", bufs=2) as nrmp,
                tc.tile_pool(name="ystage", bufs=2) as yp,
                tc.tile_pool(name="sps", bufs=3, space="PSUM") as sps,
                tc.tile_pool(name="ops", bufs=2, space="PSUM") as ops,
                tc.tile_pool(name="dps", bufs=1, space="PSUM") as dps,
                tc.tile_pool(name="yps", bufs=2, space="PSUM") as yps,
            ):
                wo_t = []
                for cc in range(HPC):
                    t = wop.tile([P, D], f32r, tag=f"wo{cc}", name=f"wo{cc}")
                    nc.sync.dma_start(out=t[:], in_=Wo3[:, cc, :])
                    wo_t.append(t)

                def emit_outproj_mm(oqb, ot_prev, slot, state):
                    # One out-projection matmul of block `oqb`, interleaved
                    # into the attention stream as PE filler during exp waits.
                    # slot runs 0..63 across the 4 heads of the next block.
                    rc, eb, hc = slot // 16, (slot // 4) % 4, slot % 4
                    if hc == 0:
                        state["ps_y"] = yps.tile([P, 512], f32, tag="y",
                                                 name=f"yps{oqb}_{rc}_{eb}")
                    nc.tensor.matmul(
                        state["ps_y"][:],
                        ot_prev[hc][:, rc * P:(rc + 1) * P],
                        wo_t[hc][:, eb * 512:(eb + 1) * 512],
                        start=(hc == 0), stop=(hc == HPC - 1),
                    )
                    if hc == HPC - 1:
                        if eb == 0:
                            state["ytile"] = yp.tile([P, D], f32, tag="y",
                                                     name=f"yt{oqb}_{rc}")
                        nc.vector.tensor_copy(
                            state["ytile"][:, eb * 512:(eb + 1) * 512],
                            state["ps_y"][:],
                        )
                        if eb == 3:
                            row0 = oqb * 512 + rc * P
                            nc.sync.dma_start(
                                out=Y[row0:row0 + P, :], in_=state["ytile"][:]
                            )

                def attention_block(qb, prev_ot):
                    qs = slice(qb * 512, (qb + 1) * 512)
                    state = {}
                    ot_tiles = []
                    for h in range(HPC):
                        ps_o = ops.tile([P, 512], f32, tag="o")
                        den = nrmp.tile([P, 512], f32r, tag="den")
                        den2 = nrmp.tile([P, 512], f32r, tag="den2")
                        for kc in range(S // P):
                            ps_s = sps.tile([P, 512], f32, tag="s")
                            nc.tensor.matmul(
                                ps_s[:], Kt[h][:, kc * P:(kc + 1) * P],
                                Qt[h][:, qs], start=True, stop=True,
                            )
                            if prev_ot is not None:
                                emit_outproj_mm(qb - 1, prev_ot,
                                                h * (S // P) + kc, state)
                            pt = ptp.tile([P, 512], f32r, tag="pt")
                            nc.scalar.activation(
                                pt[:], ps_s[:],
                                mybir.ActivationFunctionType.Exp,
                                scale=INV_SQRT_DK,
                            )
                            nc.tensor.matmul(
                                ps_o[:], Vt[kc][:, h * P:(h + 1) * P], pt[:],
                                start=(kc == 0), stop=(kc == S // P - 1),
                            )
                            dtile = den if kc % 2 == 0 else den2
                            if kc < 2:
                                nc.vector.tensor_copy(dtile[:], pt[:])
                            else:
                                nc.vector.tensor_add(dtile[:], dtile[:], pt[:])
                        nc.vector.tensor_add(den[:], den[:], den2[:])
                        ps_d = dps.tile([P, 512], f32, tag="d")
                        nc.tensor.matmul(
                            ps_d[:], ones_t[:, 0:P], den[:], start=True, stop=True,
                        )
                        rbc = nrmp.tile([P, 512], f32, tag="rbc")
                        nc.vector.reciprocal_approx_fast(rbc[:], ps_d[:])
                        ot = otp.tile([P, 512], f32r, tag=f"ot{h}",
                                      name=f"ot{qb}_{h}")
                        nc.vector.tensor_mul(ot[:], ps_o[:], rbc[:])
                        ot_tiles.append(ot)
                    return ot_tiles

                prev_ot = None
                for qb in range(NRB):
                    prev_ot = attention_block(qb, prev_ot)
                # Tail: last block's output projection, un-interleaved.
                state = {}
                for slot in range(64):
                    emit_outproj_mm(NRB - 1, prev_ot, slot, state)

    nc.compile()
    return nc


def kernel(q, k, v, Wq, bq, Wk, bk, Wv, bv, Wo, bo):
    global LAST_EXEC_NS
    q = np.asarray(q, dtype=np.float32)
    k = np.asarray(k, dtype=np.float32)
    v = np.asarray(v, dtype=np.float32)
    Wq = np.asarray(Wq, dtype=np.float32)
    Wk = np.asarray(Wk, dtype=np.float32)
    Wv = np.asarray(Wv, dtype=np.float32)
    Wo = np.asarray(Wo, dtype=np.float32)
    bq = np.asarray(bq, dtype=np.float32)
    bk = np.asarray(bk, dtype=np.float32)
    bv = np.asarray(bv, dtype=np.float32)
    bo = np.asarray(bo, dtype=np.float32)

    if "nc" not in _CACHE:
        _CACHE["nc"] = _build()
    nc = _CACHE["nc"]

    qTs = [np.ascontiguousarray(q[b].T) for b in range(B)]
    kTs = [np.ascontiguousarray(k[b].T) for b in range(B)]
    vTs = [np.ascontiguousarray(v[b].T) for b in range(B)]

    in_maps = []
    for c in range(N_CORES):
        b = c // (N_CORES // B)
        g = c % (N_CORES // B)
        cs = slice(g * CW, (g + 1) * CW)
        in_maps.append({
            "qT": qTs[b], "kT": kTs[b], "vT": vTs[b],
            "Wq": np.ascontiguousarray(Wq[:, cs]),
            "Wk": np.ascontiguousarray(Wk[:, cs]),
            "Wv": np.ascontiguousarray(Wv[:, cs]),
            "Wo": np.ascontiguousarray(Wo[cs, :]),
            "bq": np.ascontiguousarray(bq[cs])[None, :],
            "bk": np.ascontiguousarray(bk[cs])[None, :],
            "bv": np.ascontiguousarray(bv[cs])[None, :],
            "ones": _ONES,
        })

    _CACHE["last_in_maps"] = in_maps
    res = run_bass_kernel_spmd(nc, in_maps, core_ids=list(range(N_CORES)))
    LAST_EXEC_NS = res.exec_time_ns

    gpb = N_CORES // B
    out = np.empty((B, S, D), dtype=np.float32)
    for b in range(B):
        acc = res.results[b * gpb]["Y"].astype(np.float32)
        for g in range(1, gpb):
            acc = acc + res.results[b * gpb + g]["Y"]
        out[b] = acc + bo[None, :]
    return out
